# revision 21
# baseline (speedup 1.0000x reference)
"""CMC-V2 loss kernel for 8 Trainium2 NeuronCores (Bass/Tile).

Math
----
The reference loss decomposes into:
  - 9 NT-Xent contrastive terms. For pair (A, B) with row-normalized
    embeddings Z = [An; Bn] (N=4096 rows, D=512), the per-row sim matrix is
    sim = (Zn @ Zn.T)/0.2 = 5*cos.  Since rows are unit-norm, sim[i,i] = 5.0
    is the exact row max, so
        lse_i (diag excluded) = 5 + log(S_i - 1),  S_i = sum_j exp(5*cos_ij - 5)
    and sum_i pos_i = 10 * sum_i cos(An_i, Bn_i).
    per-pair loss = 5 + (1/4096) sum_i log(S_i - 1) - (10/4096) sum_i cos_i
  - 12 cosine-embedding terms: 1 - (1/2048) sum_i cos_i.
  Total constant: 9*5 + 12 = 57.

Sharding
--------
Data-parallel over 8 cores with a static SPMD program: core c receives every
input rolled by -256*c rows, so its shard is always rows [0:256) of each
matrix (matmul weights cannot take dynamic offsets). Each core:
  - normalizes all 12 half-matrices (bf16) and transposes them on the PE
    (identity matmul -> PSUM -> ScalarE/DVE copy) into ZnT layout
    [128 part = d%128, 4 = d//128, 2048 = sample],
  - computes its 512 Gram rows per pair (lhsT = its 256-row shard of A and B)
    against all 4096 columns; ScalarE applies exp(5x-5) with a fused
    per-row accumulate; log(S-1) summed on-chip,
  - computes its 256-row shard of the 21 row-dot (cosine) sums,
  - returns partial sums in a [128, 4] f32 tensor.
Host sums the 8 partials and applies the closed-form combination.
"""

import numpy as np
import ml_dtypes
from contextlib import ExitStack

from concourse import bass, bacc, tile, mybir
from concourse.bass_utils import run_bass_kernel_spmd

BF16 = mybir.dt.bfloat16
FP8 = mybir.dt.float8e4
F32 = mybir.dt.float32
AF = mybir.ActivationFunctionType
ALU = mybir.AluOpType

# fp8 variant: Gram matmuls in fp8e4m3 with DoubleRow (2 MACs/cell/cycle).
# Normalized rows are pre-scaled by 16 so fp8 sees values ~N(0, 0.71^2);
# the Gram then yields 256*cos and the exp scale becomes 5/256.
USE_FP8 = True
FP8_SCALE = 16.0

B = 2048          # batch
DH = 512          # half feature dim
N_CORES = 8
R = B // N_CORES  # 256 rows per core shard
NT = B // 128     # 16 row tiles per half-matrix
KC = DH // 128    # 4 contraction chunks
CBW = 512         # column block width
CB = B // CBW     # 4 col blocks per matrix

NAMES = ["f1_m0", "f1_m1", "f1_m2", "f2_m0", "f2_m1", "f2_m2"]

# contrastive pairs as ((f, h), (f, h)); h: 0 = shared, 1 = private
PAIRS_S1 = [((0, 0), (1, 0)), ((0, 0), (2, 0)), ((1, 0), (2, 0))]
PAIRS_S2 = [((3, 0), (4, 0)), ((3, 0), (5, 0)), ((4, 0), (5, 0))]
PAIRS_P = [((0, 1), (3, 1)), ((1, 1), (4, 1)), ((2, 1), (5, 1))]
ORTHO_V1 = [((0, 0), (0, 1)), ((1, 0), (1, 1)), ((2, 0), (2, 1)),
            ((0, 1), (1, 1)), ((0, 1), (2, 1)), ((1, 1), (2, 1))]
ORTHO_V2 = [((3, 0), (3, 1)), ((4, 0), (4, 1)), ((5, 0), (5, 1)),
            ((3, 1), (4, 1)), ((3, 1), (5, 1)), ((4, 1), (5, 1))]

N_SLOTS = 9 * 4   # 9 pairs x 4 M-tiles of 128 Gram rows each
N_DOTS = 21       # 9 contrastive + 12 ortho row-dot sums



def build_program(use_fp8=USE_FP8, repeat=1, loads_on="sync",
                  psum_banks=2, psum_bufs=3, timing_mode="full",
                  transpose_via="pe_d", squares_on="vector", copy_mod=0):
    # Restrict ACT table selection to the one set containing BOTH exp and ln
    # (greedy per-op selection would otherwise thrash exp_and_others <->
    # natural_log, ~1.3us per reload, serialized on ScalarE).
    if not getattr(bacc, "_ant_act_tables_patched", False):
        _orig_tables = bacc.get_activation_tables

        def _patched(arch):
            tabs = _orig_tables(arch)
            return {k: (v if k == "natural_log_exp_and_others" else set())
                    for k, v in tabs.items()}

        bacc.get_activation_tables = _patched
        bacc._ant_act_tables_patched = True

    nc = bacc.Bacc(
        "TRN2",
        target_bir_lowering=False,
        debug=False,
        enable_asserts=False,
        num_devices=N_CORES,
    )
    ffs = [nc.dram_tensor(n, [B, 2 * DH], BF16, kind="ExternalInput").ap()
           for n in NAMES]
    out_dram = nc.dram_tensor("part", [128, 4], F32, kind="ExternalOutput").ap()

    n_sub = 2 * B // (psum_banks * CBW)     # psum tiles per (pair, mtile)
    cb_per = psum_banks                     # 512-col blocks per psum tile

    with tile.TileContext(nc) as tc, ExitStack() as ctx:
        znt_pool = ctx.enter_context(tc.tile_pool(name="zntp", bufs=9))
        x_pool = ctx.enter_context(tc.tile_pool(name="xp", bufs=6))
        zn_pool = ctx.enter_context(tc.tile_pool(name="znp", bufs=4))
        vscr_pool = ctx.enter_context(tc.tile_pool(name="vscrp", bufs=3))
        escr_pool = ctx.enter_context(tc.tile_pool(name="escrp", bufs=3))
        nrm_pool = ctx.enter_context(tc.tile_pool(name="nrmp", bufs=3))
        sab_pool = ctx.enter_context(tc.tile_pool(name="sabp", bufs=4))
        acc_pool = ctx.enter_context(tc.tile_pool(name="accp", bufs=1))
        psum_pool = ctx.enter_context(
            tc.tile_pool(name="psump", bufs=psum_bufs, space="PSUM"))

        load_eng = {"gpsimd": nc.gpsimd, "scalar": nc.scalar,
                    "sync": nc.sync}[loads_on]
        # xbar transposes alternate across both HWDGE rings (SP + ACT) so
        # their trigger/ucode cost is not serialized on one ring.
        tr_engs = [nc.sync, nc.scalar]
        tr_i = [0]

        def tr_dma(**kwargs):
            tr_engs[tr_i[0] % 2].dma_start(**kwargs)
            tr_i[0] += 1

        biasm5 = acc_pool.tile([128, 1], F32, tag="biasm5", name="biasm5")
        nc.gpsimd.memset(biasm5[:], -5.0)
        scl0 = FP8_SCALE if use_fp8 else 1.0
        biasln = acc_pool.tile([128, 1], F32, tag="biasln", name="biasln")
        nc.gpsimd.memset(biasln[:], float(np.log(scl0)))
        if transpose_via in ("pe", "pe_d"):
            # identity for PE transposes: ident[p, j] = (j == p)
            ident = acc_pool.tile([128, 128], BF16, tag="ident", name="ident")
            iota_r = acc_pool.tile([128, 128], F32, tag="iota_r", name="iota_r")
            iota_p = acc_pool.tile([128, 1], F32, tag="iota_p", name="iota_p")
            nc.gpsimd.iota(iota_r[:], pattern=[[1, 128]], base=0,
                           channel_multiplier=0,
                           allow_small_or_imprecise_dtypes=True)
            nc.gpsimd.iota(iota_p[:], pattern=[[0, 1]], base=0,
                           channel_multiplier=1,
                           allow_small_or_imprecise_dtypes=True)
            nc.vector.tensor_scalar(
                out=ident[:], in0=iota_r[:], scalar1=iota_p[:, 0:1],
                scalar2=None, op0=ALU.is_equal)
        cp_i = [0]
        sm1 = acc_pool.tile([128, N_SLOTS], F32, tag="sm1", name="sm1")
        dots_all = acc_pool.tile([128, N_DOTS], F32, tag="dots", name="dots_all")
        logv = acc_pool.tile([128, N_SLOTS], F32, tag="logv", name="logv")
        part = acc_pool.tile([128, 4], F32, tag="part", name="part_sb")

        znt = {}

        rep_ctx = tc.For_i(0, repeat, 1) if repeat > 1 else None
        if rep_ctx is not None:
            rep_ctx.__enter__()

        def build_ff(f):
            """Load ff tensor f once per row-tile; normalize both halves and
            store transposed (bf16 Zn, or 16*Zn cast to fp8e4m3).
            znt[(f,h)][p, c, j] = Zn_h[j, c*128 + p].

            transpose_via="pe_d": normalization is folded into the PE
            transpose by streaming D = diag(16*rinv) instead of the
            identity (the raw data tile is the stationary operand either
            way), eliminating the DVE normalize pass entirely.  norms/rinv
            layout is t-major (col = 2*t + h) so ln/exp batch over both
            halves of a 4-tile group in one [128, 8] activation each."""
            zts = []
            for h in range(2):
                # fp8: the PSUM-drain copy casts bf16->fp8 directly, so the
                # znt tile is fp8 from the start (no intermediate + cast pass)
                zts.append(znt_pool.tile(
                    [128, KC, B], FP8 if use_fp8 else BF16, tag="znt",
                    name=f"znt{f}_{h}"))
            norms = nrm_pool.tile([128, 2 * NT], F32, tag="norms", name=f"nrm{f}")
            lgn = nrm_pool.tile([128, 2 * NT], F32, tag="lgn", name=f"lgn{f}")
            rinv = nrm_pool.tile([128, 2 * NT], F32, tag="rinv", name=f"rinv{f}")
            pe_d = transpose_via == "pe_d"
            scl = FP8_SCALE if use_fp8 else 1.0
            ncol = (lambda t, h: 2 * t + h) if pe_d else (
                lambda t, h: h * NT + t)
            for g in range(NT // 4):
                xts = []
                for u in range(2):
                    # one 3D DMA covers two 128-row tiles: [128, 2, 1024]
                    xt = x_pool.tile([128, 2, 2 * DH], BF16, tag="xt",
                                     name=f"xt{f}_{g}_{u}")
                    base = (4 * g + 2 * u) * 128
                    load_eng.dma_start(
                        out=xt[:],
                        in_=ffs[f][base:base + 256, :].rearrange(
                            "(tt p) c -> p tt c", p=128))
                    xts.append(xt)
                for i, t in enumerate(range(4 * g, 4 * g + 4)):
                    xv = xts[i // 2][:, i % 2, :]
                    for h in range(2):
                        sq = vscr_pool.tile([128, DH], F32, tag="vscr",
                                            name=f"sq{f}_{h}_{t}")
                        sq_eng = (nc.gpsimd if squares_on == "gpsimd"
                                  else nc.vector)
                        c0 = ncol(t, h)
                        sq_eng.scalar_tensor_tensor(
                            out=sq[:], in0=xv[:, h * DH:(h + 1) * DH],
                            scalar=1.0, in1=xv[:, h * DH:(h + 1) * DH],
                            op0=ALU.mult, op1=ALU.mult,
                            accum_out=norms[:, c0:c0 + 1])
                if pe_d:
                    # rinv = scl*ss**-0.5 = exp(-0.5*ln(ss) + ln(scl)); both
                    # funcs live in the natural_log_exp_and_others table set.
                    cs = slice(8 * g, 8 * g + 8)
                    nc.scalar.activation(lgn[:, cs], norms[:, cs], AF.Ln)
                    nc.scalar.activation(rinv[:, cs], lgn[:, cs], AF.Exp,
                                         scale=-0.5, bias=biasln[:])
                else:
                    for h in range(2):
                        cs = slice(h * NT + 4 * g, h * NT + 4 * g + 4)
                        nc.scalar.activation(lgn[:, cs], norms[:, cs], AF.Ln)
                        nc.scalar.activation(rinv[:, cs], lgn[:, cs], AF.Exp,
                                             scale=-0.5)
                for i, t in enumerate(range(4 * g, 4 * g + 4)):
                    xv = xts[i // 2][:, i % 2, :]
                    for h in range(2):
                        if pe_d:
                            c0 = ncol(t, h)
                            dmat = zn_pool.tile([128, 128], BF16, tag="zn",
                                                name=f"dm{f}_{h}_{t}")
                            nc.vector.tensor_scalar_mul(
                                out=dmat[:], in0=ident[:],
                                scalar1=rinv[:, c0:c0 + 1])
                            tp = psum_pool.tile([128, KC, 128], BF16,
                                                tag="tpp", bufs=2,
                                                name=f"tp{f}_{h}_{t}")
                            for c in range(KC):
                                nc.tensor.matmul(
                                    tp[:, c, :],
                                    xv[:, h * DH + c * 128:
                                       h * DH + (c + 1) * 128],
                                    dmat[:], start=True, stop=True)
                            dst = zts[h][:, :, t * 128:(t + 1) * 128]
                            if copy_mod and cp_i[0] % copy_mod == 0:
                                nc.scalar.copy(dst, tp[:, :, :])
                            else:
                                nc.vector.tensor_copy(dst, tp[:, :, :])
                            cp_i[0] += 1
                            continue
                        zn = zn_pool.tile([128, DH], BF16, tag="zn",
                                          name=f"zn{f}_{h}_{t}")
                        if use_fp8:
                            nc.vector.tensor_scalar(
                                out=zn[:], in0=xv[:, h * DH:(h + 1) * DH],
                                scalar1=rinv[:, h * NT + t:h * NT + t + 1],
                                scalar2=FP8_SCALE, op0=ALU.mult, op1=ALU.mult)
                        else:
                            nc.vector.tensor_scalar_mul(
                                out=zn[:], in0=xv[:, h * DH:(h + 1) * DH],
                                scalar1=rinv[:, h * NT + t:h * NT + t + 1])
                        if transpose_via == "xbar":
                            tr_dma(
                                out=zts[h][:, :, t * 128:(t + 1) * 128],
                                in_=zn[:], transpose=True)
                        else:
                            # PE transpose into PSUM, then one strided
                            # PSUM->SBUF copy alternating ScalarE/DVE.
                            tp = psum_pool.tile([128, KC, 128], BF16,
                                                tag="tpp", bufs=2,
                                                name=f"tp{f}_{h}_{t}")
                            for c in range(KC):
                                nc.tensor.transpose(
                                    tp[:, c, :], zn[:, c * 128:(c + 1) * 128],
                                    ident[:])
                            dst = zts[h][:, :, t * 128:(t + 1) * 128]
                            # copy_mod: every copy_mod-th copy on ScalarE,
                            # rest on DVE (0 = all DVE)
                            if copy_mod and cp_i[0] % copy_mod == 0:
                                nc.scalar.copy(dst, tp[:, :, :])
                            else:
                                nc.vector.tensor_copy(dst, tp[:, :, :])
                            cp_i[0] += 1
            for h in range(2):
                znt[(f, h)] = zts[h]

        slot_i = [0]

        def gram(A, Bm):
            """Gram rows + fused exp/rowsum for contrastive pair (A, Bm)."""
            for X in (A, Bm):          # lhsT source (core 256-row shard)
                for mt in range(2):    # two 128-row M tiles
                    si = slot_i[0]
                    sab = sab_pool.tile([128, n_sub], F32, tag="sab",
                                        name=f"sab{si}")
                    for ridx, RH in enumerate((A, Bm)):   # rhs matrix
                        for sub in range(n_sub // 2):
                            ps = psum_pool.tile(
                                [128, cb_per, CBW], F32, tag="gram",
                                name=f"ps{si}_{ridx}_{sub}")
                            for cbl in range(cb_per):
                                cb = sub * cb_per + cbl
                                if use_fp8:
                                    for q in range(KC // 2):
                                        nc.tensor.matmul(
                                            ps[:, cbl, :],
                                            znt[X][:, 2 * q:2 * q + 2,
                                                   mt * 128:(mt + 1) * 128],
                                            znt[RH][:, 2 * q:2 * q + 2,
                                                    cb * CBW:(cb + 1) * CBW],
                                            perf_mode=mybir.MatmulPerfMode.DoubleRow,
                                            start=(q == 0),
                                            stop=(q == KC // 2 - 1))
                                else:
                                    for kc in range(KC):
                                        nc.tensor.matmul(
                                            ps[:, cbl, :],
                                            znt[X][:, kc,
                                                   mt * 128:(mt + 1) * 128],
                                            znt[RH][:, kc,
                                                    cb * CBW:(cb + 1) * CBW],
                                            start=(kc == 0),
                                            stop=(kc == KC - 1))
                            es = escr_pool.tile([128, cb_per, CBW], BF16,
                                                tag="escr",
                                                name=f"es{si}_{ridx}_{sub}")
                            exp_scale = (5.0 / (FP8_SCALE * FP8_SCALE)
                                         if use_fp8 else 5.0)
                            col = ridx * (n_sub // 2) + sub
                            nc.scalar.activation(
                                es[:], ps[:], AF.Exp, bias=biasm5[:],
                                scale=exp_scale,
                                accum_out=sab[:, col:col + 1])
                    # sm1[:, slot] = sum(sab) - 1
                    scr2 = sab_pool.tile([128, n_sub], F32, tag="scr2",
                                         name=f"scr2_{si}")
                    nc.vector.tensor_scalar(
                        out=scr2[:], in0=sab[:], scalar1=-1.0 / n_sub,
                        scalar2=None, op0=ALU.add, op1=ALU.add,
                        accum_out=sm1[:, si:si + 1])
                    slot_i[0] += 1

        def dots(col, X, Y):
            """dots_all[:, col] = per-partition sum over the core 256-row
            shard of <Zn_X[i], Zn_Y[i]> (row-wise cosines)."""
            o = vscr_pool.tile([128, KC, R], F32, tag="vscr", name=f"do{col}")
            dscale = 1.0 / (FP8_SCALE * FP8_SCALE) if use_fp8 else 1.0
            nc.vector.scalar_tensor_tensor(
                out=o[:], in0=znt[X][:, :, 0:R], scalar=dscale,
                in1=znt[Y][:, :, 0:R], op0=ALU.mult, op1=ALU.mult,
                accum_out=dots_all[:, col:col + 1])

        if timing_mode == "grams":
            # timing probe: skip builds; map the 12 halves onto 9 shared
            # tiles (timing-equivalent op stream, garbage data).
            shared = []
            for s in range(9):
                t = znt_pool.tile([128, KC, B], FP8 if use_fp8 else BF16,
                                  tag="znt", name=f"znts{s}")
                nc.vector.memset(t[:, :, 0:2], 0.0)
                shared.append(t)
            for f in range(6):
                for h in range(2):
                    znt[(f, h)] = shared[(2 * f + h) % 9]
            def build_ff(f):
                pass
        elif timing_mode == "builds":
            nc.vector.memset(sm1[:], 1.0)
            def gram(A, Bm):
                pass

        # Interleaved emission: each gram/dot is emitted as soon as the
        # matrices it needs are built, so no engine queue convoys behind an
        # unrelated phase.  dots_all cols: 0..8 contrastive in the order
        # (s1 x3, private x3, s2 x3); 9..20 ortho (v1 then v2).
        build_ff(0)
        build_ff(1)
        gram((0, 0), (1, 0))
        dots(0, (0, 0), (1, 0))
        dots(9, (0, 0), (0, 1)); dots(10, (1, 0), (1, 1))
        dots(12, (0, 1), (1, 1))
        build_ff(2)
        gram((0, 0), (2, 0)); gram((1, 0), (2, 0))
        dots(1, (0, 0), (2, 0)); dots(2, (1, 0), (2, 0))
        dots(11, (2, 0), (2, 1))
        dots(13, (0, 1), (2, 1)); dots(14, (1, 1), (2, 1))
        build_ff(3)
        gram((0, 1), (3, 1))
        dots(3, (0, 1), (3, 1))
        dots(15, (3, 0), (3, 1))
        build_ff(4)
        gram((1, 1), (4, 1)); gram((3, 0), (4, 0))
        dots(4, (1, 1), (4, 1)); dots(6, (3, 0), (4, 0))
        dots(16, (4, 0), (4, 1)); dots(18, (3, 1), (4, 1))
        build_ff(5)
        gram((2, 1), (5, 1)); gram((3, 0), (5, 0)); gram((4, 0), (5, 0))
        dots(5, (2, 1), (5, 1)); dots(7, (3, 0), (5, 0))
        dots(8, (4, 0), (5, 0))
        dots(17, (5, 0), (5, 1)); dots(19, (3, 1), (5, 1))
        dots(20, (4, 1), (5, 1))

        # ---- epilogue ----
        nc.scalar.activation(logv[:], sm1[:], AF.Ln)
        nc.vector.memset(part[:], 0.0)
        nc.vector.tensor_reduce(part[:, 0:1], logv[:], axis=mybir.AxisListType.X,
                                op=ALU.add)
        nc.vector.tensor_reduce(part[:, 1:2], dots_all[:, 0:9],
                                axis=mybir.AxisListType.X, op=ALU.add)
        nc.vector.tensor_reduce(part[:, 2:3], dots_all[:, 9:21],
                                axis=mybir.AxisListType.X, op=ALU.add)
        nc.sync.dma_start(out=out_dram, in_=part[:])

        if rep_ctx is not None:
            rep_ctx.__exit__(None, None, None)

    nc.compile()
    return nc


# ---------------------------------------------------------------------------
# Step B: uniform pair-ownership resharding.
#
# Each core owns ONE full contrastive pair (all 4096 Gram rows) plus a
# 512-row share of the 9th pair; it builds only the 4 half-matrices it
# needs (vs 12), and computes all 21 row-dot (cosine) terms on its 256-row
# shard of every half from cheap row-major shard inputs.  The single SPMD
# program is identical across cores; per-core in_maps bind different
# matrices into the A/B/C/D slots (and roll the 9th pair so the template's
# "rows [0:512)" selects a distinct row block per core).
# ---------------------------------------------------------------------------

# half id k = 2*f + h (f = tensor index in NAMES, h: 0 = shared, 1 = private)
PAIRS_B = [(0, 2), (0, 4), (2, 4),       # S1 triangle (f1 shared halves)
           (6, 8), (6, 10), (8, 10),     # S2 triangle (f2 shared halves)
           (1, 7), (3, 9)]               # P0, P1 (private, across views)
PAIR8_B = (5, 11)                        # P2 (2p, 5p), row-split across cores
# contrastive edges (9) then ortho edges (12), in k-space
EDGES_C = [(0, 2), (0, 4), (2, 4), (6, 8), (6, 10), (8, 10),
           (1, 7), (3, 9), (5, 11)]
EDGES_O = [(0, 1), (2, 3), (4, 5), (1, 3), (1, 5), (3, 5),
           (6, 7), (8, 9), (10, 11), (7, 9), (7, 11), (9, 11)]
NT_B = 16         # 128-row tiles per half matrix
RT8 = 4           # pair-8 row tiles per core


def build_program_b(use_fp8=USE_FP8, repeat=1, loads_on="sync",
                    psum_bufs=3, es_dtype="bf16", interleave=2):
    if not getattr(bacc, "_ant_act_tables_patched", False):
        _orig_tables = bacc.get_activation_tables

        def _patched(arch):
            tabs = _orig_tables(arch)
            return {k: (v if k == "natural_log_exp_and_others" else set())
                    for k, v in tabs.items()}

        bacc.get_activation_tables = _patched
        bacc._ant_act_tables_patched = True

    nc = bacc.Bacc(
        "TRN2",
        target_bir_lowering=False,
        debug=False,
        enable_asserts=False,
        num_devices=N_CORES,
    )
    halves = {n: nc.dram_tensor(n, [B, DH], BF16, kind="ExternalInput").ap()
              for n in ("ha", "hb", "hc", "hd")}
    shards = [nc.dram_tensor(f"sh{k}", [R, DH], BF16,
                             kind="ExternalInput").ap() for k in range(12)]
    out_dram = nc.dram_tensor("part", [128, 4], F32, kind="ExternalOutput").ap()

    with tile.TileContext(nc) as tc, ExitStack() as ctx:
        znt_pool = ctx.enter_context(tc.tile_pool(name="zntp", bufs=16))
        x_pool = ctx.enter_context(tc.tile_pool(name="xp", bufs=4))
        d_pool = ctx.enter_context(tc.tile_pool(name="dp", bufs=4))
        vscr_pool = ctx.enter_context(tc.tile_pool(name="vscrp", bufs=3))
        escr_pool = ctx.enter_context(tc.tile_pool(name="escrp", bufs=3))
        nrm_pool = ctx.enter_context(tc.tile_pool(name="nrmp", bufs=4))
        sab_pool = ctx.enter_context(tc.tile_pool(name="sabp", bufs=4))
        sh_pool = ctx.enter_context(tc.tile_pool(name="shp", bufs=14))
        acc_pool = ctx.enter_context(tc.tile_pool(name="accp", bufs=1))
        psum_pool = ctx.enter_context(
            tc.tile_pool(name="psump", bufs=psum_bufs, space="PSUM"))

        load_eng = {"gpsimd": nc.gpsimd, "scalar": nc.scalar,
                    "sync": nc.sync}[loads_on]
        scl = FP8_SCALE if use_fp8 else 1.0
        esd = {"bf16": BF16, "fp8": FP8}[es_dtype]

        biasm5 = acc_pool.tile([128, 1], F32, tag="biasm5", name="biasm5")
        nc.gpsimd.memset(biasm5[:], -5.0)
        biasln = acc_pool.tile([128, 1], F32, tag="biasln", name="biasln")
        nc.gpsimd.memset(biasln[:], float(np.log(scl)))
        ident = acc_pool.tile([128, 128], BF16, tag="ident", name="ident")
        iota_r = acc_pool.tile([128, 128], F32, tag="iota_r", name="iota_r")
        iota_p = acc_pool.tile([128, 1], F32, tag="iota_p", name="iota_p")
        nc.gpsimd.iota(iota_r[:], pattern=[[1, 128]], base=0,
                       channel_multiplier=0,
                       allow_small_or_imprecise_dtypes=True)
        nc.gpsimd.iota(iota_p[:], pattern=[[0, 1]], base=0,
                       channel_multiplier=1,
                       allow_small_or_imprecise_dtypes=True)
        nc.vector.tensor_scalar(
            out=ident[:], in0=iota_r[:], scalar1=iota_p[:, 0:1],
            scalar2=None, op0=ALU.is_equal)

        n_slots = 2 * NT_B + RT8
        sm1 = acc_pool.tile([128, n_slots], F32, tag="sm1", name="sm1")
        dots_all = acc_pool.tile([128, 42], F32, tag="dots", name="dots_all")
        logv = acc_pool.tile([128, n_slots], F32, tag="logv", name="logv")
        part = acc_pool.tile([128, 4], F32, tag="part", name="part_sb")

        znt = {}

        rep_ctx = tc.For_i(0, repeat, 1) if repeat > 1 else None
        if rep_ctx is not None:
            rep_ctx.__enter__()

        nrm_tiles = {}

        def build_half_group(x, g, squares_on="scalar"):
            """Build column-group g (512 samples) of znt[x]:
            znt[x][g][p, c, j] = scl * Xn[512*g + j, c*128 + p].
            Normalization (and the fp8 range scale) is folded into the PE
            pass: the raw 128x128 data chunk is the stationary operand and
            D = diag(scl*rinv) is the moving operand (a regular matmul —
            transpose-mode would ignore rhs values).  squares_on="scalar"
            runs the row-sum-of-squares as a Square activation on the
            (otherwise idle during builds) ScalarE."""
            norms, lgn, rinv = nrm_tiles[x]
            zt = znt_pool.tile([128, KC, CBW], FP8 if use_fp8 else BF16,
                               tag="znt", name=f"znt_{x}_{g}")
            xt = x_pool.tile([128, 4, DH], BF16, tag="xt", name=f"xt{x}_{g}")
            load_eng.dma_start(
                out=xt[:],
                in_=halves[x][g * 512:(g + 1) * 512, :].rearrange(
                    "(tt p) c -> p tt c", p=128))
            for i in range(4):
                t = 4 * g + i
                sq = vscr_pool.tile([128, DH], F32, tag="vscr",
                                    name=f"sq{x}_{t}")
                if squares_on == "scalar":
                    nc.scalar.activation(sq[:], xt[:, i, :], AF.Square,
                                         accum_out=norms[:, t:t + 1])
                else:
                    nc.vector.scalar_tensor_tensor(
                        out=sq[:], in0=xt[:, i, :], scalar=1.0,
                        in1=xt[:, i, :], op0=ALU.mult, op1=ALU.mult,
                        accum_out=norms[:, t:t + 1])
            cs = slice(4 * g, 4 * g + 4)
            nc.scalar.activation(lgn[:, cs], norms[:, cs], AF.Ln)
            nc.scalar.activation(rinv[:, cs], lgn[:, cs], AF.Exp,
                                 scale=-0.5, bias=biasln[:])
            for i in range(4):
                t = 4 * g + i
                dmat = d_pool.tile([128, 128], BF16, tag="dm",
                                   name=f"dm{x}_{t}")
                nc.vector.tensor_scalar_mul(
                    out=dmat[:], in0=ident[:], scalar1=rinv[:, t:t + 1])
                tp = psum_pool.tile([128, KC, 128], F32, tag="tpp",
                                    bufs=2, name=f"tp{x}_{t}")
                for c in range(KC):
                    nc.tensor.matmul(
                        tp[:, c, :], xt[:, i, c * 128:(c + 1) * 128],
                        dmat[:], start=True, stop=True)
                nc.vector.tensor_copy(
                    zt[:, :, i * 128:(i + 1) * 128], tp[:, :, :])
            znt[x][g] = zt

        def init_half(x):
            znt[x] = [None] * 4
            nrm_tiles[x] = (
                nrm_pool.tile([128, NT_B], F32, tag="nrm", name=f"nrm{x}"),
                nrm_pool.tile([128, NT_B], F32, tag="lgn", name=f"lgn{x}"),
                nrm_pool.tile([128, NT_B], F32, tag="rinv", name=f"ri{x}"))

        def build_two_halves(x, y, squares_on="scalar"):
            """Interleave the column-groups of two halves so early Gram
            units (which need group 0 of both) unlock as soon as possible."""
            init_half(x)
            init_half(y)
            for g in range(4):
                build_half_group(x, g, squares_on)
                build_half_group(y, g, squares_on)

        slot_i = [0]

        def gram_rows(src, mt, rx, ry):
            """One 128-row Gram slot: lhsT = znt[src] group mt//4, cols
            (mt%4)*128; rhs = all 4096 cols of znt[rx] ++ znt[ry], paired
            per column-group so unit cb only needs group cb of both halves.
            Fused exp rowsum via ScalarE accum; sm1[:, slot] = rowsum - 1
            (self-sim)."""
            si = slot_i[0]
            sab = sab_pool.tile([128, 4], F32, tag="sab", name=f"sab{si}")
            lg, lc = mt // 4, (mt % 4) * 128
            for cb in range(4):
                ps = psum_pool.tile([128, 2, CBW], F32, tag="gram",
                                    name=f"ps{si}_{cb}")
                for cbl, rh in enumerate((rx, ry)):
                    if use_fp8:
                        for q in range(KC // 2):
                            nc.tensor.matmul(
                                ps[:, cbl, :],
                                znt[src][lg][:, 2 * q:2 * q + 2,
                                             lc:lc + 128],
                                znt[rh][cb][:, 2 * q:2 * q + 2, :],
                                perf_mode=mybir.MatmulPerfMode.DoubleRow,
                                start=(q == 0), stop=(q == KC // 2 - 1))
                    else:
                        for kc in range(KC):
                            nc.tensor.matmul(
                                ps[:, cbl, :],
                                znt[src][lg][:, kc, lc:lc + 128],
                                znt[rh][cb][:, kc, :],
                                start=(kc == 0), stop=(kc == KC - 1))
                es = escr_pool.tile([128, 2, CBW], esd, tag="escr",
                                    name=f"es{si}_{cb}")
                exp_scale = (5.0 / (scl * scl) if use_fp8 else 5.0)
                nc.scalar.activation(
                    es[:], ps[:], AF.Exp, bias=biasm5[:],
                    scale=exp_scale, accum_out=sab[:, cb:cb + 1])
            scr2 = sab_pool.tile([128, 4], F32, tag="scr2", name=f"sc2_{si}")
            nc.vector.tensor_scalar(
                out=scr2[:], in0=sab[:], scalar1=-1.0 / 4, scalar2=None,
                op0=ALU.add, op1=ALU.add, accum_out=sm1[:, si:si + 1])
            slot_i[0] += 1

        sht = []
        shn = []

        def dots_norms(squares_on="scalar"):
            """Phase 1: load all 12 shard halves, row sums of squares (on
            ScalarE while it is otherwise idle), one ln + one exp."""
            snorm = nrm_pool.tile([128, 24], F32, tag="snrm", name="snorm")
            slgn = nrm_pool.tile([128, 24], F32, tag="slgn", name="slgn")
            srinv = nrm_pool.tile([128, 24], F32, tag="srinv", name="srinv")
            for k in range(12):
                sh = sh_pool.tile([128, 2, DH], BF16, tag="shr",
                                  name=f"shr{k}")
                load_eng.dma_start(
                    out=sh[:],
                    in_=shards[k][:, :].rearrange("(tt p) c -> p tt c", p=128))
                sht.append(sh)
                for r2 in range(2):
                    sq = vscr_pool.tile([128, DH], F32, tag="vscr",
                                        name=f"ssq{k}_{r2}")
                    if squares_on == "scalar":
                        nc.scalar.activation(
                            sq[:], sh[:, r2, :], AF.Square,
                            accum_out=snorm[:, 2 * k + r2:2 * k + r2 + 1])
                    else:
                        nc.vector.scalar_tensor_tensor(
                            out=sq[:], in0=sh[:, r2, :], scalar=1.0,
                            in1=sh[:, r2, :], op0=ALU.mult, op1=ALU.mult,
                            accum_out=snorm[:, 2 * k + r2:2 * k + r2 + 1])
            nc.scalar.activation(slgn[:], snorm[:], AF.Ln)
            nc.scalar.activation(srinv[:], slgn[:], AF.Exp, scale=-0.5)
            return srinv

        def dots_prescale(srinv):
            """Phase 2a: normalize shards in row layout (DVE, 4x mode)."""
            for k in range(12):
                s = sh_pool.tile([128, 2, DH], BF16, tag="shn", name=f"shn{k}")
                for r2 in range(2):
                    nc.vector.tensor_scalar_mul(
                        out=s[:, r2, :], in0=sht[k][:, r2, :],
                        scalar1=srinv[:, 2 * k + r2:2 * k + r2 + 1])
                shn.append(s)

        def dots_edges(e0, e1):
            """Phase 2b: dots_all[:, 2e+r] = sum_j shn_X[p,r,j]*shn_Y[p,r,j]
            for edges [e0, e1)."""
            for e in range(e0, e1):
                kx, ky = (EDGES_C + EDGES_O)[e]
                for r2 in range(2):
                    o = vscr_pool.tile([128, DH], F32, tag="vscr",
                                       name=f"do{e}_{r2}")
                    nc.vector.scalar_tensor_tensor(
                        out=o[:], in0=shn[kx][:, r2, :], scalar=1.0,
                        in1=shn[ky][:, r2, :], op0=ALU.mult, op1=ALU.mult,
                        accum_out=dots_all[:, 2 * e + r2:2 * e + r2 + 1])

        # ---- emission ----
        # A/B builds + shard norms first: their squares run on ScalarE
        # (idle during the DVE/PE-bound build ramp).  C/D build work and
        # the dot STTs (DVE-only by then) are sliced between gram slots so
        # ScalarE's strict-FIFO queue never holds a small activation that
        # waits on busy DVE in front of ready gram exps.
        build_two_halves("ha", "hb", squares_on="scalar")
        srinv = dots_norms(squares_on="scalar")
        own = [("ha", mt) for mt in range(NT_B)] + \
              [("hb", mt) for mt in range(NT_B)]
        init_half("hc")
        init_half("hd")
        emit_mid = {
            2: lambda: build_half_group("hc", 0, "vector"),
            4: lambda: build_half_group("hd", 0, "vector"),
            6: lambda: build_half_group("hc", 1, "vector"),
            8: lambda: build_half_group("hd", 1, "vector"),
            10: lambda: build_half_group("hc", 2, "vector"),
            12: lambda: build_half_group("hd", 2, "vector"),
            14: lambda: build_half_group("hc", 3, "vector"),
            16: lambda: build_half_group("hd", 3, "vector"),
            18: lambda: dots_prescale(srinv),
            20: lambda: dots_edges(0, 7),
            22: lambda: dots_edges(7, 14),
            24: lambda: dots_edges(14, 21),
        }
        for i, (src, mt) in enumerate(own):
            if i in emit_mid:
                emit_mid[i]()
            gram_rows(src, mt, "ha", "hb")
        for mt in range(RT8):
            gram_rows("hc", mt, "hc", "hd")

        # ---- epilogue ----
        nc.scalar.activation(logv[:], sm1[:], AF.Ln)
        nc.vector.memset(part[:], 0.0)
        nc.vector.tensor_reduce(part[:, 0:1], logv[:], axis=mybir.AxisListType.X,
                                op=ALU.add)
        nc.vector.tensor_reduce(part[:, 1:2], dots_all[:, 0:18],
                                axis=mybir.AxisListType.X, op=ALU.add)
        nc.vector.tensor_reduce(part[:, 2:3], dots_all[:, 18:42],
                                axis=mybir.AxisListType.X, op=ALU.add)
        nc.sync.dma_start(out=out_dram, in_=part[:])

        if rep_ctx is not None:
            rep_ctx.__exit__(None, None, None)

    nc.compile()
    return nc


def make_in_maps_b(inputs):
    bf = ml_dtypes.bfloat16

    def half(k):
        f, h = divmod(k, 2)
        a = np.asarray(inputs[NAMES[f]], dtype=np.float32)
        return a[:, h * DH:(h + 1) * DH]

    in_maps = []
    for c in range(N_CORES):
        m = {}
        ka, kb = PAIRS_B[c]
        m["ha"] = np.ascontiguousarray(half(ka)).astype(bf)
        m["hb"] = np.ascontiguousarray(half(kb)).astype(bf)
        k8a, k8b = PAIR8_B
        if c < 4:
            m["hc"] = np.ascontiguousarray(
                np.roll(half(k8a), -512 * c, axis=0)).astype(bf)
            m["hd"] = np.ascontiguousarray(half(k8b)).astype(bf)
        else:
            m["hc"] = np.ascontiguousarray(
                np.roll(half(k8b), -512 * (c - 4), axis=0)).astype(bf)
            m["hd"] = np.ascontiguousarray(half(k8a)).astype(bf)
        for k in range(12):
            m[f"sh{k}"] = np.ascontiguousarray(
                half(k)[R * c:R * (c + 1), :]).astype(bf)
        in_maps.append(m)
    return in_maps


def combine_b(parts):
    """parts: 8 x [128, 4] f32.  Same closed form as combine(): per pair,
    loss = 5 + mean(log(S_i - 1)) - (10/N) sum_i cos_i; ortho terms
    1 - mean(cos).  part cols: 0 = sum log(S-1), 1 = sum contrastive cos
    (over shard rows), 2 = sum ortho cos."""
    tl = tcc = toc = 0.0
    for p in parts:
        p = np.asarray(p, dtype=np.float64)
        tl += p[:, 0].sum()
        tcc += p[:, 1].sum()
        toc += p[:, 2].sum()
    n2 = float(2 * B)
    loss = (9 * 5.0 + 12.0) + tl / n2 - 10.0 * tcc / n2 - toc / float(B)
    return np.float32(loss)


USE_B = True
_PROG = None


def _get_prog():
    global _PROG
    if _PROG is None:
        _PROG = build_program_b() if USE_B else build_program()
    return _PROG


def make_in_maps(inputs):
    bf = ml_dtypes.bfloat16
    in_maps = []
    for c in range(N_CORES):
        m = {}
        for n in NAMES:
            a = np.asarray(inputs[n], dtype=np.float32)
            m[n] = np.ascontiguousarray(np.roll(a, -R * c, axis=0)).astype(bf)
        in_maps.append(m)
    return in_maps


def combine(parts):
    """parts: list of 8 [128, 4] f32 arrays -> scalar loss."""
    tl = tcc = toc = 0.0
    for p in parts:
        p = np.asarray(p, dtype=np.float64)
        tl += p[:, 0].sum()
        tcc += p[:, 1].sum()
        toc += p[:, 2].sum()
    n2 = float(2 * B)
    loss = (9 * 5.0 + 12.0) + tl / n2 - 10.0 * tcc / n2 - toc / float(B)
    return np.float32(loss)


def kernel(**inputs):
    nc = _get_prog()
    in_maps = (make_in_maps_b if USE_B else make_in_maps)(inputs)
    res = run_bass_kernel_spmd(nc, in_maps, list(range(N_CORES)))
    comb = combine_b if USE_B else combine
    return comb([res.results[c]["part"] for c in range(N_CORES)])

